# revision 16
# baseline (speedup 1.0000x reference)
"""Trainium2 Bass kernel for batched single-query attention (Luong-style).

  scores[b, t] = dec_hid[b] . enc_hid_states[b, t]      # [B, T]
  align        = softmax(scores, axis=1)
  c_t[b, d]    = sum_t align[b, t] * enc_hid_states[b, t, d]

Shapes: enc_hid_states [32, 8192, 256] f32, dec_hid [32, 256] f32.
Sharding: data-parallel over batch; 4 batches per core on 8 cores, no
cross-core communication (output rows are concatenated on the host).

Per-core pipeline (per batch, the 8 MiB enc slice is read from HBM exactly
once and kept in SBUF). Flash-attention style: each 1 MiB supertile
[128(t%128), 8(t//128), 256(d)] flows through a local softmax so every
engine is busy concurrently instead of phase-barriered:
  - DVE multiplies the supertile by a stride-0-broadcast dec vector;
    dot-product reduces split between DVE (3D tensor_reduce) and ACT
    (activation Copy + accum_out) to balance engine time
  - local max (DVE) -> GPSIMD partition all-reduce -> ACT Exp with
    bias=-m_s and fused sum-of-exp -> GPSIMD all-reduce
  - 8 accumulating PE matmuls (lhsT=probs column [128,1], rhs=enc tile
    [128,256], float32r = full-rate fp32 streaming) produce the
    supertile's partial context in PSUM
  - per batch, partials are combined with log-sum-exp weights
    w_s = exp(m_s - M): one small PE transpose + matmul, scale by 1/Z.

The kernel must avoid two environment pitfalls discovered empirically:
InstTensorTensorReduce faults this terminal's DVE (device becomes
NRT_EXEC_UNIT_UNRECOVERABLE), and the Tile kernel-tail semaphore
RANGE_CLEAR is replaced by a drain+barrier-only tail (see
_tail_no_semclear).
"""

import sys
from contextlib import ExitStack

import numpy as np

sys.path.insert(0, "/opt/trn_rl_repo")

import concourse.bacc as bacc
import concourse.bass as bass
import concourse.bass_isa as bass_isa
import concourse.mybir as mybir
import concourse.tile as tile
from concourse.bass_utils import run_bass_kernel_spmd
from concourse.tile import ScopedClock


def _tail_no_semclear(self, tick_clock, wait_clock):
    """Tile's kernel-tail normally drains, barriers, then issues a GPSIMD
    dma_reset + EVENT_SEMAPHORE_RANGE_CLEAR over every sem it allocated.
    NRT resets semaphore state between executions, so drain + barrier alone
    is sufficient under the one-shot PJRT execution used here."""
    drain_inst = self.nc.sync.drain()
    wait_clock.add_sem_waits(
        drain_inst.ins, ScopedClock({None: tick_clock.global_clock})
    )
    self.nc.all_engine_barrier()
    popped = self.nc._tile_sem_poison_stack.pop()
    assert popped is self._sem_poison
    self.nc.all_engine_barrier()


tile.TileContext._drain_and_barrier = _tail_no_semclear

B, T, D = 32, 8192, 256
N_CORES = 8
B_LOC = B // N_CORES  # 4 batches per core
P = 128               # partitions
NJ = T // P           # 64 row-tiles per batch
SUP = 8               # row-tiles per supertile (1 MiB DMA granularity)
NS = NJ // SUP        # 8 supertiles per batch
ST_BUFS = 17          # supertile slots (136 KiB/part)
DVE_REDUCE_SET = {0, 3, 6}  # supertiles reduced on DVE; rest on ACT

# float32r streams at 1 col/cycle for N>=256 (plain float32 is 4x slower).
PHASE2_DT = mybir.dt.float32r


def _build_nc():
    f32 = mybir.dt.float32
    nc = bacc.Bacc(
        "TRN2",
        target_bir_lowering=False,
        debug=False,
        enable_asserts=False,
        num_devices=N_CORES,
    )
    enc = nc.dram_tensor("enc", [B_LOC, T, D], f32, kind="ExternalInput")
    dec = nc.dram_tensor("dec", [B_LOC, D], f32, kind="ExternalInput")
    out = nc.dram_tensor("out", [B_LOC, D], f32, kind="ExternalOutput")

    enc_r = enc.ap().rearrange("b (j p) d -> b p j d", p=P)  # [B_LOC, 128, 64, 256]
    dec_ap = dec.ap()
    out_ap = out.ap()

    with tile.TileContext(nc) as tc, ExitStack() as ctx:
        st_pool = ctx.enter_context(tc.tile_pool(name="st", bufs=ST_BUFS))
        prod_pool = ctx.enter_context(tc.tile_pool(name="prod", bufs=4))
        dec_pool = ctx.enter_context(tc.tile_pool(name="decb", bufs=2))
        small = ctx.enter_context(tc.tile_pool(name="small", bufs=4))
        outp = ctx.enter_context(tc.tile_pool(name="outp", bufs=2))
        psum_c = ctx.enter_context(tc.tile_pool(name="psc", bufs=3, space="PSUM"))
        psum_w = ctx.enter_context(tc.tile_pool(name="psw", bufs=2, space="PSUM"))

        # one-time constants
        ident1 = small.tile([1, 1], f32, tag="ident1")
        nc.vector.memset(ident1, 1.0)

        for b in range(B_LOC):
            # dec[b] replicated across partitions and 8 j-groups
            dec_bc = dec_pool.tile([P, D], f32, tag="dec_bc")
            dslice = dec_ap[b : b + 1, :]
            dec_src = bass.AP(
                tensor=dslice.tensor,
                offset=dslice.offset,
                ap=[[0, P], [1, D]],
            )
            nc.sync.dma_start(out=dec_bc, in_=dec_src)
            dec_bc3 = dec_bc[:, :].rearrange("p (u d) -> p u d", u=1).to_broadcast(
                [P, SUP, D]
            )

            sts = []
            for s in range(NS):
                st = st_pool.tile([P, SUP, D], PHASE2_DT, tag="st")
                nc.sync.dma_start(
                    out=st,
                    in_=enc_r[b, :, s * SUP : (s + 1) * SUP, :].bitcast(PHASE2_DT),
                )
                sts.append(st)

            # per-supertile stats (column s of each is constant across
            # partitions after the GPSIMD all-reduce) and context partials
            SM = small.tile([P, NS], f32, tag="SM")    # local maxes
            SZ = small.tile([P, NS], f32, tag="SZ")    # local sum-of-exp
            Csup = small.tile([NS, D], f32, tag="Csup")  # partial contexts

            for s in range(NS):
                # scores for this supertile
                S = small.tile([P, SUP], f32, tag="S")
                prod = prod_pool.tile([P, SUP, D], f32, tag="prod")
                nc.vector.tensor_tensor(
                    out=prod,
                    in0=sts[s].bitcast(f32),
                    in1=dec_bc3,
                    op=mybir.AluOpType.mult,
                )
                on_dve = s in DVE_REDUCE_SET
                if on_dve:
                    nc.vector.tensor_reduce(
                        out=S,
                        in_=prod,
                        axis=mybir.AxisListType.X,
                        op=mybir.AluOpType.add,
                    )
                else:
                    for jj in range(SUP):
                        junk = small.tile([P, D], f32, tag="junk")
                        nc.scalar.activation(
                            out=junk,
                            in_=prod[:, jj, :],
                            func=mybir.ActivationFunctionType.Copy,
                            bias=0.0,
                            scale=1.0,
                            accum_out=S[:, jj : jj + 1],
                        )

                # local softmax stats
                m_loc = small.tile([P, 1], f32, tag="m_loc")
                nc.vector.tensor_reduce(
                    out=m_loc, in_=S, axis=mybir.AxisListType.X,
                    op=mybir.AluOpType.max,
                )
                nc.gpsimd.partition_all_reduce(
                    SM[:, s : s + 1], m_loc, channels=P,
                    reduce_op=bass_isa.ReduceOp.max,
                )
                negm = small.tile([P, 1], f32, tag="negm")
                nc.gpsimd.tensor_scalar_mul(
                    out=negm, in0=SM[:, s : s + 1], scalar1=-1.0
                )

                probs = small.tile([P, SUP], PHASE2_DT, tag="probs")
                sz_loc = small.tile([P, 1], f32, tag="sz_loc")
                nc.scalar.activation(
                    out=probs,
                    in_=S,
                    func=mybir.ActivationFunctionType.Exp,
                    bias=negm,
                    scale=1.0,
                    accum_out=sz_loc,
                )
                nc.gpsimd.partition_all_reduce(
                    SZ[:, s : s + 1], sz_loc, channels=P,
                    reduce_op=bass_isa.ReduceOp.add,
                )

                # partial context for this supertile
                ps = psum_c.tile([1, D], f32, tag="ps")
                for jj in range(SUP):
                    nc.tensor.matmul(
                        out=ps,
                        lhsT=probs[:, jj : jj + 1],
                        rhs=sts[s][:, jj, :],
                        start=(jj == 0),
                        stop=(jj == SUP - 1),
                    )
                # stage the partial at partition 0 (engines can't start at
                # partition s), then DMA it into row s of Csup
                csb = small.tile([1, D], f32, tag="csb")
                if on_dve:
                    nc.scalar.copy(out=csb, in_=ps)
                else:
                    nc.vector.tensor_copy(out=csb, in_=ps)
                nc.sync.dma_start(out=Csup[s : s + 1, :], in_=csb)

            # combine: c = sum_s exp(m_s - M) * Csup[s] / sum_s exp(m_s - M) * Z_s
            M = small.tile([1, 1], f32, tag="M")
            nc.vector.tensor_reduce(
                out=M, in_=SM[0:1, :], axis=mybir.AxisListType.X,
                op=mybir.AluOpType.max,
            )
            negM = small.tile([1, 1], f32, tag="negM")
            nc.gpsimd.tensor_scalar_mul(out=negM, in0=M, scalar1=-1.0)
            w_row = small.tile([1, NS], f32, tag="w_row")
            nc.scalar.activation(
                out=w_row,
                in_=SM[0:1, :],
                func=mybir.ActivationFunctionType.Exp,
                bias=negM,
                scale=1.0,
            )
            wz = small.tile([1, NS], f32, tag="wz")
            nc.vector.tensor_tensor(
                out=wz, in0=w_row, in1=SZ[0:1, :], op=mybir.AluOpType.mult
            )
            Z = small.tile([1, 1], f32, tag="Z")
            nc.vector.tensor_reduce(
                out=Z, in_=wz, axis=mybir.AxisListType.X, op=mybir.AluOpType.add
            )
            invz = small.tile([1, 1], f32, tag="invz")
            nc.vector.reciprocal(out=invz, in_=Z)

            # w as a column via PE transpose, then c_hat = w^T @ Csup
            ps_w = psum_w.tile([NS, 1], f32, tag="ps_w")
            nc.tensor.transpose(out=ps_w, in_=w_row, identity=ident1)
            w_col = small.tile([NS, 1], f32, tag="w_col")
            nc.vector.tensor_copy(out=w_col, in_=ps_w)
            ps_c = psum_w.tile([1, D], f32, tag="ps_chat")
            nc.tensor.matmul(
                out=ps_c, lhsT=w_col, rhs=Csup, start=True, stop=True
            )

            c_sb = outp.tile([1, D], f32, tag="c_sb")
            nc.vector.tensor_scalar_mul(out=c_sb, in0=ps_c, scalar1=invz)
            nc.sync.dma_start(out=out_ap[b : b + 1, :], in_=c_sb)

    nc.compile()
    return nc


_NC_CACHE = None


def _get_nc():
    global _NC_CACHE
    if _NC_CACHE is None:
        _NC_CACHE = _build_nc()
    return _NC_CACHE


def run_on_cores(enc_np: np.ndarray, dec_np: np.ndarray, trace: bool = False):
    """Returns (out [32, 256] f32, BassKernelResults)."""
    nc = _get_nc()
    in_maps = [
        {
            "enc": np.ascontiguousarray(enc_np[c * B_LOC : (c + 1) * B_LOC]),
            "dec": np.ascontiguousarray(dec_np[c * B_LOC : (c + 1) * B_LOC]),
        }
        for c in range(N_CORES)
    ]
    res = run_bass_kernel_spmd(nc, in_maps, list(range(N_CORES)), trace=trace)
    out = np.concatenate([r["out"] for r in res.results], axis=0)
    return out.astype(np.float32), res


def kernel(enc_hid_states, dec_hid):
    enc_np = np.asarray(enc_hid_states, dtype=np.float32)
    dec_np = np.asarray(dec_hid, dtype=np.float32)
    out, _ = run_on_cores(enc_np, dec_np, trace=False)
    return out


# revision 19
# speedup vs baseline: 1.2388x; 1.2388x over previous
"""Trainium2 Bass kernel for batched single-query attention (Luong-style).

  scores[b, t] = dec_hid[b] . enc_hid_states[b, t]      # [B, T]
  align        = softmax(scores, axis=1)
  c_t[b, d]    = sum_t align[b, t] * enc_hid_states[b, t, d]

Shapes: enc_hid_states [32, 8192, 256] f32, dec_hid [32, 256] f32.
Sharding: data-parallel over batch; 4 batches per core on 8 cores, no
cross-core communication (output rows are concatenated on the host).

Per-core pipeline (per batch, the 8 MiB enc slice is read from HBM exactly
once and kept in SBUF). Flash-attention style: each 1 MiB supertile
[128(t%128), 8(t//128), 256(d)] flows through a local softmax so every
engine is busy concurrently instead of phase-barriered:
  - DVE multiplies the supertile by a stride-0-broadcast dec vector;
    dot-product reduces split between DVE (3D tensor_reduce) and ACT
    (activation Copy + accum_out) to balance engine time
  - local max (DVE) -> GPSIMD partition all-reduce -> ACT Exp with
    bias=-m_s and fused sum-of-exp -> GPSIMD all-reduce
  - 8 accumulating PE matmuls (lhsT=probs column [128,1], rhs=enc tile
    [128,256], float32r = full-rate fp32 streaming) produce the
    supertile's partial context in PSUM
  - per batch, partials are combined with log-sum-exp weights
    w_s = exp(m_s - M): one small PE transpose + matmul, scale by 1/Z.

The kernel must avoid two environment pitfalls discovered empirically:
InstTensorTensorReduce faults this terminal's DVE (device becomes
NRT_EXEC_UNIT_UNRECOVERABLE), and the Tile kernel-tail semaphore
RANGE_CLEAR is replaced by a drain+barrier-only tail (see
_tail_no_semclear).
"""

import sys
from contextlib import ExitStack

import numpy as np

sys.path.insert(0, "/opt/trn_rl_repo")

import concourse.bacc as bacc
import concourse.bass as bass
import concourse.bass_isa as bass_isa
import concourse.mybir as mybir
import concourse.tile as tile
from concourse.bass_utils import run_bass_kernel_spmd
from concourse.tile import ScopedClock


def _tail_no_semclear(self, tick_clock, wait_clock):
    """Tile's kernel-tail normally drains, barriers, then issues a GPSIMD
    dma_reset + EVENT_SEMAPHORE_RANGE_CLEAR over every sem it allocated.
    NRT resets semaphore state between executions, so drain + barrier alone
    is sufficient under the one-shot PJRT execution used here."""
    drain_inst = self.nc.sync.drain()
    wait_clock.add_sem_waits(
        drain_inst.ins, ScopedClock({None: tick_clock.global_clock})
    )
    self.nc.all_engine_barrier()
    popped = self.nc._tile_sem_poison_stack.pop()
    assert popped is self._sem_poison
    self.nc.all_engine_barrier()


tile.TileContext._drain_and_barrier = _tail_no_semclear

B, T, D = 32, 8192, 256
N_CORES = 8
B_LOC = B // N_CORES  # 4 batches per core
P = 128               # partitions
NJ = T // P           # 64 row-tiles per batch
SUP = 8               # row-tiles per supertile (1 MiB DMA granularity)
NS = NJ // SUP        # 8 supertiles per batch
ST_BUFS = 26          # supertile slots, fp16 => 104 KiB/part (3+ batches)
DVE_REDUCE_SET = {0, 2, 4, 6}  # supertiles reduced on DVE; rest on ACT

# enc/probs live as fp16 on-chip: the GPSIMD casting DMA halves SBUF
# footprint, the all-2-byte DVE multiply runs in 2x_1p mode (0.5x cycles),
# and fp16 PE matmuls stream at 1 col/cycle like bf16.
PHASE2_DT = mybir.dt.float16


def _build_nc():
    f32 = mybir.dt.float32
    nc = bacc.Bacc(
        "TRN2",
        target_bir_lowering=False,
        debug=False,
        enable_asserts=False,
        num_devices=N_CORES,
    )
    enc = nc.dram_tensor("enc", [B_LOC, T, D], f32, kind="ExternalInput")
    dec = nc.dram_tensor("dec", [B_LOC, D], f32, kind="ExternalInput")
    out = nc.dram_tensor("out", [B_LOC, D], f32, kind="ExternalOutput")

    enc_r = enc.ap().rearrange("b (j p) d -> b p j d", p=P)  # [B_LOC, 128, 64, 256]
    dec_ap = dec.ap()
    out_ap = out.ap()

    with tile.TileContext(nc) as tc, ExitStack() as ctx:
        st_pool = ctx.enter_context(tc.tile_pool(name="st", bufs=ST_BUFS))
        prod_pool = ctx.enter_context(tc.tile_pool(name="prod", bufs=4))
        dec_pool = ctx.enter_context(tc.tile_pool(name="decb", bufs=2))
        small = ctx.enter_context(tc.tile_pool(name="small", bufs=4))
        outp = ctx.enter_context(tc.tile_pool(name="outp", bufs=2))
        psum_c = ctx.enter_context(tc.tile_pool(name="psc", bufs=3, space="PSUM"))
        psum_w = ctx.enter_context(tc.tile_pool(name="psw", bufs=1, space="PSUM"))

        # one-time constants
        ident1 = small.tile([1, 1], f32, tag="ident1")
        nc.vector.memset(ident1, 1.0)
        ones_col = small.tile([P, 1], f32, tag="ones_col")
        nc.vector.memset(ones_col, 1.0)

        for b in range(B_LOC):
            # dec[b] replicated across partitions and 8 j-groups
            dec_bc = dec_pool.tile([P, D], PHASE2_DT, tag="dec_bc")
            dslice = dec_ap[b : b + 1, :]
            dec_src = bass.AP(
                tensor=dslice.tensor,
                offset=dslice.offset,
                ap=[[0, P], [1, D]],
            )
            nc.gpsimd.dma_start(out=dec_bc, in_=dec_src)
            dec_bc3 = dec_bc[:, :].rearrange("p (u d) -> p u d", u=1).to_broadcast(
                [P, SUP, D]
            )

            sts = []
            for s in range(NS):
                st = st_pool.tile([P, SUP, D], PHASE2_DT, tag="st")
                nc.gpsimd.dma_start(
                    out=st,
                    in_=enc_r[b, :, s * SUP : (s + 1) * SUP, :],
                )
                sts.append(st)

            # per-supertile stats (column s of each is constant across
            # partitions after the GPSIMD all-reduce) and context partials
            SM = small.tile([P, NS], f32, tag="SM")    # local maxes
            SZ = small.tile([P, NS], f32, tag="SZ")    # per-partition sum-of-exp
            Csup = small.tile([NS, D], f32, tag="Csup")  # partial contexts

            for s in range(NS):
                # scores for this supertile
                S = small.tile([P, SUP], f32, tag="S")
                prod = prod_pool.tile([P, SUP, D], PHASE2_DT, tag="prod")
                nc.vector.tensor_tensor(
                    out=prod,
                    in0=sts[s],
                    in1=dec_bc3,
                    op=mybir.AluOpType.mult,
                )
                on_dve = s in DVE_REDUCE_SET
                if on_dve:
                    nc.vector.tensor_reduce(
                        out=S,
                        in_=prod,
                        axis=mybir.AxisListType.X,
                        op=mybir.AluOpType.add,
                    )
                else:
                    for jj in range(SUP):
                        junk = small.tile([P, D], PHASE2_DT, tag="junk")
                        nc.scalar.activation(
                            out=junk,
                            in_=prod[:, jj, :],
                            func=mybir.ActivationFunctionType.Copy,
                            bias=0.0,
                            scale=1.0,
                            accum_out=S[:, jj : jj + 1],
                        )

                # local softmax stats
                m_loc = small.tile([P, 1], f32, tag="m_loc")
                nc.vector.tensor_reduce(
                    out=m_loc, in_=S, axis=mybir.AxisListType.X,
                    op=mybir.AluOpType.max,
                )
                nc.gpsimd.partition_all_reduce(
                    SM[:, s : s + 1], m_loc, channels=P,
                    reduce_op=bass_isa.ReduceOp.max,
                )
                negm = small.tile([P, 1], f32, tag="negm")
                nc.gpsimd.tensor_scalar_mul(
                    out=negm, in0=SM[:, s : s + 1], scalar1=-1.0
                )

                probs = small.tile([P, SUP], PHASE2_DT, tag="probs")
                nc.scalar.activation(
                    out=probs,
                    in_=S,
                    func=mybir.ActivationFunctionType.Exp,
                    bias=negm,
                    scale=1.0,
                    accum_out=SZ[:, s : s + 1],
                )

                # partial context for this supertile
                ps = psum_c.tile([1, D], f32, tag="ps")
                for jj in range(SUP):
                    nc.tensor.matmul(
                        out=ps,
                        lhsT=probs[:, jj : jj + 1],
                        rhs=sts[s][:, jj, :],
                        start=(jj == 0),
                        stop=(jj == SUP - 1),
                    )
                # stage the partial at partition 0 (engines can't start at
                # partition s), then DMA it into row s of Csup
                csb = small.tile([1, D], f32, tag="csb")
                nc.vector.tensor_copy(out=csb, in_=ps)
                nc.sync.dma_start(out=Csup[s : s + 1, :], in_=csb)

            # combine: c = sum_s exp(m_s - M) * Csup[s] / sum_s exp(m_s - M) * Z_s
            M = small.tile([1, 1], f32, tag="M")
            nc.vector.tensor_reduce(
                out=M, in_=SM[0:1, :], axis=mybir.AxisListType.X,
                op=mybir.AluOpType.max,
            )
            negM = small.tile([1, 1], f32, tag="negM")
            nc.gpsimd.tensor_scalar_mul(out=negM, in0=M, scalar1=-1.0)
            w_row = small.tile([1, NS], f32, tag="w_row")
            nc.scalar.activation(
                out=w_row,
                in_=SM[0:1, :],
                func=mybir.ActivationFunctionType.Exp,
                bias=negM,
                scale=1.0,
            )
            # Z_col[s] = sum_p SZ[p, s] via PE, then Z = w . Z_col
            ps_z = psum_w.tile([NS, 1], f32, tag="ps_z")
            nc.tensor.matmul(
                out=ps_z, lhsT=SZ, rhs=ones_col, start=True, stop=True
            )
            z_col = small.tile([NS, 1], f32, tag="z_col")
            nc.vector.tensor_copy(out=z_col, in_=ps_z)

            # w as a column via PE transpose, then c_hat = w^T @ Csup
            ps_w = psum_w.tile([NS, 1], f32, tag="ps_w")
            nc.tensor.transpose(out=ps_w, in_=w_row, identity=ident1)
            w_col = small.tile([NS, 1], f32, tag="w_col")
            nc.vector.tensor_copy(out=w_col, in_=ps_w)
            ps_zf = psum_w.tile([1, 1], f32, tag="ps_zf")
            nc.tensor.matmul(
                out=ps_zf, lhsT=w_col, rhs=z_col, start=True, stop=True
            )
            invz = small.tile([1, 1], f32, tag="invz")
            nc.vector.reciprocal(out=invz, in_=ps_zf)
            ps_c = psum_w.tile([1, D], f32, tag="ps_chat")
            nc.tensor.matmul(
                out=ps_c, lhsT=w_col, rhs=Csup, start=True, stop=True
            )

            c_sb = outp.tile([1, D], f32, tag="c_sb")
            nc.vector.tensor_scalar_mul(out=c_sb, in0=ps_c, scalar1=invz)
            nc.sync.dma_start(out=out_ap[b : b + 1, :], in_=c_sb)

    nc.compile()
    return nc


_NC_CACHE = None


def _get_nc():
    global _NC_CACHE
    if _NC_CACHE is None:
        _NC_CACHE = _build_nc()
    return _NC_CACHE


def run_on_cores(enc_np: np.ndarray, dec_np: np.ndarray, trace: bool = False):
    """Returns (out [32, 256] f32, BassKernelResults)."""
    nc = _get_nc()
    in_maps = [
        {
            "enc": np.ascontiguousarray(enc_np[c * B_LOC : (c + 1) * B_LOC]),
            "dec": np.ascontiguousarray(dec_np[c * B_LOC : (c + 1) * B_LOC]),
        }
        for c in range(N_CORES)
    ]
    res = run_bass_kernel_spmd(nc, in_maps, list(range(N_CORES)), trace=trace)
    out = np.concatenate([r["out"] for r in res.results], axis=0)
    return out.astype(np.float32), res


def kernel(enc_hid_states, dec_hid):
    enc_np = np.asarray(enc_hid_states, dtype=np.float32)
    dec_np = np.asarray(dec_hid, dtype=np.float32)
    out, _ = run_on_cores(enc_np, dec_np, trace=False)
    return out


# revision 21
# speedup vs baseline: 1.2723x; 1.0270x over previous
"""Trainium2 Bass kernel for batched single-query attention (Luong-style).

  scores[b, t] = dec_hid[b] . enc_hid_states[b, t]      # [B, T]
  align        = softmax(scores, axis=1)
  c_t[b, d]    = sum_t align[b, t] * enc_hid_states[b, t, d]

Shapes: enc_hid_states [32, 8192, 256] f32, dec_hid [32, 256] f32.
Sharding: data-parallel over batch; 4 batches per core on 8 cores, no
cross-core communication (output rows are concatenated on the host).

Per-core pipeline (per batch, the 8 MiB enc slice is read from HBM exactly
once and kept in SBUF). Flash-attention style: each 1 MiB supertile
[128(t%128), 8(t//128), 256(d)] flows through a local softmax so every
engine is busy concurrently instead of phase-barriered:
  - DVE multiplies the supertile by a stride-0-broadcast dec vector;
    dot-product reduces split between DVE (3D tensor_reduce) and ACT
    (activation Copy + accum_out) to balance engine time
  - local max (DVE) -> GPSIMD partition all-reduce -> ACT Exp with
    bias=-m_s and fused sum-of-exp -> GPSIMD all-reduce
  - 8 accumulating PE matmuls (lhsT=probs column [128,1], rhs=enc tile
    [128,256], both fp16 = full-rate streaming) produce the supertile's
    partial context in PSUM
  - per batch, partials are combined with log-sum-exp weights
    w_s = exp(m_s - M): small PE transpose + matmuls (including the
    deferred cross-partition sum-of-exp reduce), scale by 1/Z.

enc/dec are cast f32->fp16 by the GPSIMD casting DMA on load: HBM traffic
is unchanged (32 MiB/core, read once) but the DVE multiply qualifies for
the all-2-byte 2x_1p perf mode (0.5x cycles) and SBUF footprint halves,
allowing 3 batches in flight. Cost: ~4e-3 relative error (vs ~8e-4 for
the all-f32 variant preserved in kernel_v2_flash_f32.py).

The kernel must avoid two environment pitfalls discovered empirically:
InstTensorTensorReduce faults this terminal's DVE (device becomes
NRT_EXEC_UNIT_UNRECOVERABLE), and the Tile kernel-tail semaphore
RANGE_CLEAR is replaced by a drain+barrier-only tail (see
_tail_no_semclear).
"""

import sys
from contextlib import ExitStack

import numpy as np

sys.path.insert(0, "/opt/trn_rl_repo")

import concourse.bacc as bacc
import concourse.bass as bass
import concourse.bass_isa as bass_isa
import concourse.mybir as mybir
import concourse.tile as tile
from concourse.bass_utils import run_bass_kernel_spmd
from concourse.tile import ScopedClock


def _tail_no_semclear(self, tick_clock, wait_clock):
    """Tile's kernel-tail normally drains, barriers, then issues a GPSIMD
    dma_reset + EVENT_SEMAPHORE_RANGE_CLEAR over every sem it allocated.
    NRT resets semaphore state between executions, so drain + barrier alone
    is sufficient under the one-shot PJRT execution used here."""
    drain_inst = self.nc.sync.drain()
    wait_clock.add_sem_waits(
        drain_inst.ins, ScopedClock({None: tick_clock.global_clock})
    )
    self.nc.all_engine_barrier()
    popped = self.nc._tile_sem_poison_stack.pop()
    assert popped is self._sem_poison
    self.nc.all_engine_barrier()


tile.TileContext._drain_and_barrier = _tail_no_semclear

B, T, D = 32, 8192, 256
N_CORES = 8
B_LOC = B // N_CORES  # 4 batches per core
P = 128               # partitions
NJ = T // P           # 64 row-tiles per batch
SUP = 8               # row-tiles per supertile (1 MiB DMA granularity)
NS = NJ // SUP        # 8 supertiles per batch
ST_BUFS = 30          # supertile slots, fp16 => 120 KiB/part (~4 batches)
DVE_REDUCE_SET = {0, 2, 4, 6}  # supertiles reduced on DVE; rest on ACT

# enc/probs live as fp16 on-chip: the GPSIMD casting DMA halves SBUF
# footprint, the all-2-byte DVE multiply runs in 2x_1p mode (0.5x cycles),
# and fp16 PE matmuls stream at 1 col/cycle like bf16.
PHASE2_DT = mybir.dt.float16


def _build_nc():
    f32 = mybir.dt.float32
    nc = bacc.Bacc(
        "TRN2",
        target_bir_lowering=False,
        debug=False,
        enable_asserts=False,
        num_devices=N_CORES,
    )
    enc = nc.dram_tensor("enc", [B_LOC, T, D], f32, kind="ExternalInput")
    dec = nc.dram_tensor("dec", [B_LOC, D], f32, kind="ExternalInput")
    out = nc.dram_tensor("out", [B_LOC, D], f32, kind="ExternalOutput")

    enc_r = enc.ap().rearrange("b (j p) d -> b p j d", p=P)  # [B_LOC, 128, 64, 256]
    dec_ap = dec.ap()
    out_ap = out.ap()

    with tile.TileContext(nc) as tc, ExitStack() as ctx:
        st_pool = ctx.enter_context(tc.tile_pool(name="st", bufs=ST_BUFS))
        prod_pool = ctx.enter_context(tc.tile_pool(name="prod", bufs=6))
        dec_pool = ctx.enter_context(tc.tile_pool(name="decb", bufs=2))
        small = ctx.enter_context(tc.tile_pool(name="small", bufs=6))
        outp = ctx.enter_context(tc.tile_pool(name="outp", bufs=2))
        psum_c = ctx.enter_context(tc.tile_pool(name="psc", bufs=4, space="PSUM"))
        psum_w = ctx.enter_context(tc.tile_pool(name="psw", bufs=1, space="PSUM"))

        # one-time constants
        ident1 = small.tile([1, 1], f32, tag="ident1")
        nc.vector.memset(ident1, 1.0)
        ones_col = small.tile([P, 1], f32, tag="ones_col")
        nc.vector.memset(ones_col, 1.0)

        for b in range(B_LOC):
            # dec[b] replicated across partitions and 8 j-groups
            dec_bc = dec_pool.tile([P, D], PHASE2_DT, tag="dec_bc")
            dslice = dec_ap[b : b + 1, :]
            dec_src = bass.AP(
                tensor=dslice.tensor,
                offset=dslice.offset,
                ap=[[0, P], [1, D]],
            )
            nc.gpsimd.dma_start(out=dec_bc, in_=dec_src)
            dec_bc3 = dec_bc[:, :].rearrange("p (u d) -> p u d", u=1).to_broadcast(
                [P, SUP, D]
            )

            sts = []
            for s in range(NS):
                st = st_pool.tile([P, SUP, D], PHASE2_DT, tag="st")
                nc.gpsimd.dma_start(
                    out=st,
                    in_=enc_r[b, :, s * SUP : (s + 1) * SUP, :],
                )
                sts.append(st)

            # per-supertile stats (column s of each is constant across
            # partitions after the GPSIMD all-reduce) and context partials
            SM = small.tile([P, NS], f32, tag="SM")    # local maxes
            SZ = small.tile([P, NS], f32, tag="SZ")    # per-partition sum-of-exp
            Csup = small.tile([NS, D], f32, tag="Csup")  # partial contexts

            for s in range(NS):
                # scores for this supertile
                S = small.tile([P, SUP], f32, tag="S")
                prod = prod_pool.tile([P, SUP, D], PHASE2_DT, tag="prod")
                nc.vector.tensor_tensor(
                    out=prod,
                    in0=sts[s],
                    in1=dec_bc3,
                    op=mybir.AluOpType.mult,
                )
                on_dve = s in DVE_REDUCE_SET
                if on_dve:
                    nc.vector.tensor_reduce(
                        out=S,
                        in_=prod,
                        axis=mybir.AxisListType.X,
                        op=mybir.AluOpType.add,
                    )
                else:
                    for jj in range(SUP):
                        junk = small.tile([P, D], PHASE2_DT, tag="junk")
                        nc.scalar.activation(
                            out=junk,
                            in_=prod[:, jj, :],
                            func=mybir.ActivationFunctionType.Copy,
                            bias=0.0,
                            scale=1.0,
                            accum_out=S[:, jj : jj + 1],
                        )

                # local softmax stats
                m_loc = small.tile([P, 1], f32, tag="m_loc")
                nc.vector.tensor_reduce(
                    out=m_loc, in_=S, axis=mybir.AxisListType.X,
                    op=mybir.AluOpType.max,
                )
                nc.gpsimd.partition_all_reduce(
                    SM[:, s : s + 1], m_loc, channels=P,
                    reduce_op=bass_isa.ReduceOp.max,
                )
                negm = small.tile([P, 1], f32, tag="negm")
                nc.gpsimd.tensor_scalar_mul(
                    out=negm, in0=SM[:, s : s + 1], scalar1=-1.0
                )

                probs = small.tile([P, SUP], PHASE2_DT, tag="probs")
                nc.scalar.activation(
                    out=probs,
                    in_=S,
                    func=mybir.ActivationFunctionType.Exp,
                    bias=negm,
                    scale=1.0,
                    accum_out=SZ[:, s : s + 1],
                )

                # partial context for this supertile
                ps = psum_c.tile([1, D], f32, tag="ps")
                for jj in range(SUP):
                    nc.tensor.matmul(
                        out=ps,
                        lhsT=probs[:, jj : jj + 1],
                        rhs=sts[s][:, jj, :],
                        start=(jj == 0),
                        stop=(jj == SUP - 1),
                    )
                # stage the partial at partition 0 (engines can't start at
                # partition s), then DMA it into row s of Csup
                csb = small.tile([1, D], f32, tag="csb")
                nc.vector.tensor_copy(out=csb, in_=ps)
                nc.sync.dma_start(out=Csup[s : s + 1, :], in_=csb)

            # combine: c = sum_s exp(m_s - M) * Csup[s] / sum_s exp(m_s - M) * Z_s
            M = small.tile([1, 1], f32, tag="M")
            nc.vector.tensor_reduce(
                out=M, in_=SM[0:1, :], axis=mybir.AxisListType.X,
                op=mybir.AluOpType.max,
            )
            negM = small.tile([1, 1], f32, tag="negM")
            nc.gpsimd.tensor_scalar_mul(out=negM, in0=M, scalar1=-1.0)
            w_row = small.tile([1, NS], f32, tag="w_row")
            nc.scalar.activation(
                out=w_row,
                in_=SM[0:1, :],
                func=mybir.ActivationFunctionType.Exp,
                bias=negM,
                scale=1.0,
            )
            # Z_col[s] = sum_p SZ[p, s] via PE, then Z = w . Z_col
            ps_z = psum_w.tile([NS, 1], f32, tag="ps_z")
            nc.tensor.matmul(
                out=ps_z, lhsT=SZ, rhs=ones_col, start=True, stop=True
            )
            z_col = small.tile([NS, 1], f32, tag="z_col")
            nc.vector.tensor_copy(out=z_col, in_=ps_z)

            # w as a column via PE transpose, then c_hat = w^T @ Csup
            ps_w = psum_w.tile([NS, 1], f32, tag="ps_w")
            nc.tensor.transpose(out=ps_w, in_=w_row, identity=ident1)
            w_col = small.tile([NS, 1], f32, tag="w_col")
            nc.vector.tensor_copy(out=w_col, in_=ps_w)
            ps_zf = psum_w.tile([1, 1], f32, tag="ps_zf")
            nc.tensor.matmul(
                out=ps_zf, lhsT=w_col, rhs=z_col, start=True, stop=True
            )
            invz = small.tile([1, 1], f32, tag="invz")
            nc.vector.reciprocal(out=invz, in_=ps_zf)
            ps_c = psum_w.tile([1, D], f32, tag="ps_chat")
            nc.tensor.matmul(
                out=ps_c, lhsT=w_col, rhs=Csup, start=True, stop=True
            )

            c_sb = outp.tile([1, D], f32, tag="c_sb")
            nc.vector.tensor_scalar_mul(out=c_sb, in0=ps_c, scalar1=invz)
            nc.sync.dma_start(out=out_ap[b : b + 1, :], in_=c_sb)

    nc.compile()
    return nc


_NC_CACHE = None


def _get_nc():
    global _NC_CACHE
    if _NC_CACHE is None:
        _NC_CACHE = _build_nc()
    return _NC_CACHE


def run_on_cores(enc_np: np.ndarray, dec_np: np.ndarray, trace: bool = False):
    """Returns (out [32, 256] f32, BassKernelResults)."""
    nc = _get_nc()
    in_maps = [
        {
            "enc": np.ascontiguousarray(enc_np[c * B_LOC : (c + 1) * B_LOC]),
            "dec": np.ascontiguousarray(dec_np[c * B_LOC : (c + 1) * B_LOC]),
        }
        for c in range(N_CORES)
    ]
    res = run_bass_kernel_spmd(nc, in_maps, list(range(N_CORES)), trace=trace)
    out = np.concatenate([r["out"] for r in res.results], axis=0)
    return out.astype(np.float32), res


def kernel(enc_hid_states, dec_hid):
    enc_np = np.asarray(enc_hid_states, dtype=np.float32)
    dec_np = np.asarray(dec_hid, dtype=np.float32)
    out, _ = run_on_cores(enc_np, dec_np, trace=False)
    return out


# revision 22
# speedup vs baseline: 1.2764x; 1.0033x over previous
"""Trainium2 Bass kernel for batched single-query attention (Luong-style).

  scores[b, t] = dec_hid[b] . enc_hid_states[b, t]      # [B, T]
  align        = softmax(scores, axis=1)
  c_t[b, d]    = sum_t align[b, t] * enc_hid_states[b, t, d]

Shapes: enc_hid_states [32, 8192, 256] f32, dec_hid [32, 256] f32.
Sharding: data-parallel over batch; 4 batches per core on 8 cores, no
cross-core communication (output rows are concatenated on the host).

Per-core pipeline (per batch, the 8 MiB enc slice is read from HBM exactly
once and kept in SBUF). Flash-attention style: each 1 MiB supertile
[128(t%128), 8(t//128), 256(d)] flows through a local softmax so every
engine is busy concurrently instead of phase-barriered:
  - DVE multiplies the supertile by a stride-0-broadcast dec vector;
    dot-product reduces split between DVE (3D tensor_reduce) and ACT
    (activation Copy + accum_out) to balance engine time
  - local max (DVE) -> GPSIMD partition all-reduce -> ACT Exp with
    bias=-m_s and fused sum-of-exp -> GPSIMD all-reduce
  - 8 accumulating PE matmuls (lhsT=probs column [128,1], rhs=enc tile
    [128,256], both fp16 = full-rate streaming) produce the supertile's
    partial context in PSUM
  - per batch, partials are combined with log-sum-exp weights
    w_s = exp(m_s - M): small PE transpose + matmuls (including the
    deferred cross-partition sum-of-exp reduce), scale by 1/Z.

enc/dec are cast f32->fp16 by the GPSIMD casting DMA on load: HBM traffic
is unchanged (32 MiB/core, read once) but the DVE multiply qualifies for
the all-2-byte 2x_1p perf mode (0.5x cycles) and SBUF footprint halves,
allowing 3 batches in flight. Cost: ~4e-3 relative error (vs ~8e-4 for
the all-f32 variant preserved in kernel_v2_flash_f32.py).

The kernel must avoid two environment pitfalls discovered empirically:
InstTensorTensorReduce faults this terminal's DVE (device becomes
NRT_EXEC_UNIT_UNRECOVERABLE), and the Tile kernel-tail semaphore
RANGE_CLEAR is replaced by a drain+barrier-only tail (see
_tail_no_semclear).
"""

import sys
from contextlib import ExitStack

import numpy as np

sys.path.insert(0, "/opt/trn_rl_repo")

import concourse.bacc as bacc
import concourse.bass as bass
import concourse.bass_isa as bass_isa
import concourse.mybir as mybir
import concourse.tile as tile
from concourse.bass_utils import run_bass_kernel_spmd
from concourse.tile import ScopedClock


def _tail_no_semclear(self, tick_clock, wait_clock):
    """Tile's kernel-tail normally drains, barriers, then issues a GPSIMD
    dma_reset + EVENT_SEMAPHORE_RANGE_CLEAR over every sem it allocated.
    NRT resets semaphore state between executions, so drain + barrier alone
    is sufficient under the one-shot PJRT execution used here."""
    drain_inst = self.nc.sync.drain()
    wait_clock.add_sem_waits(
        drain_inst.ins, ScopedClock({None: tick_clock.global_clock})
    )
    self.nc.all_engine_barrier()
    popped = self.nc._tile_sem_poison_stack.pop()
    assert popped is self._sem_poison


tile.TileContext._drain_and_barrier = _tail_no_semclear

B, T, D = 32, 8192, 256
N_CORES = 8
B_LOC = B // N_CORES  # 4 batches per core
P = 128               # partitions
NJ = T // P           # 64 row-tiles per batch
SUP = 8               # row-tiles per supertile (1 MiB DMA granularity)
NS = NJ // SUP        # 8 supertiles per batch
ST_BUFS = 30          # supertile slots, fp16 => 120 KiB/part (~4 batches)
DVE_REDUCE_SET = {0, 2, 4, 6}  # supertiles reduced on DVE; rest on ACT

# enc/probs live as fp16 on-chip: the GPSIMD casting DMA halves SBUF
# footprint, the all-2-byte DVE multiply runs in 2x_1p mode (0.5x cycles),
# and fp16 PE matmuls stream at 1 col/cycle like bf16.
PHASE2_DT = mybir.dt.float16


def _build_nc():
    f32 = mybir.dt.float32
    nc = bacc.Bacc(
        "TRN2",
        target_bir_lowering=False,
        debug=False,
        enable_asserts=False,
        num_devices=N_CORES,
    )
    enc = nc.dram_tensor("enc", [B_LOC, T, D], f32, kind="ExternalInput")
    dec = nc.dram_tensor("dec", [B_LOC, D], f32, kind="ExternalInput")
    out = nc.dram_tensor("out", [B_LOC, D], f32, kind="ExternalOutput")

    enc_r = enc.ap().rearrange("b (j p) d -> b p j d", p=P)  # [B_LOC, 128, 64, 256]
    dec_ap = dec.ap()
    out_ap = out.ap()

    with tile.TileContext(nc) as tc, ExitStack() as ctx:
        st_pool = ctx.enter_context(tc.tile_pool(name="st", bufs=ST_BUFS))
        prod_pool = ctx.enter_context(tc.tile_pool(name="prod", bufs=8))
        dec_pool = ctx.enter_context(tc.tile_pool(name="decb", bufs=2))
        small = ctx.enter_context(tc.tile_pool(name="small", bufs=8))
        outp = ctx.enter_context(tc.tile_pool(name="outp", bufs=2))
        psum_c = ctx.enter_context(tc.tile_pool(name="psc", bufs=4, space="PSUM"))
        psum_w = ctx.enter_context(tc.tile_pool(name="psw", bufs=1, space="PSUM"))

        # one-time constants
        ident1 = small.tile([1, 1], f32, tag="ident1")
        nc.vector.memset(ident1, 1.0)
        ones_col = small.tile([P, 1], f32, tag="ones_col")
        nc.vector.memset(ones_col, 1.0)

        for b in range(B_LOC):
            # dec[b] replicated across partitions and 8 j-groups
            dec_bc = dec_pool.tile([P, D], PHASE2_DT, tag="dec_bc")
            dslice = dec_ap[b : b + 1, :]
            dec_src = bass.AP(
                tensor=dslice.tensor,
                offset=dslice.offset,
                ap=[[0, P], [1, D]],
            )
            nc.gpsimd.dma_start(out=dec_bc, in_=dec_src)
            dec_bc3 = dec_bc[:, :].rearrange("p (u d) -> p u d", u=1).to_broadcast(
                [P, SUP, D]
            )

            sts = []
            for s in range(NS):
                st = st_pool.tile([P, SUP, D], PHASE2_DT, tag="st")
                nc.gpsimd.dma_start(
                    out=st,
                    in_=enc_r[b, :, s * SUP : (s + 1) * SUP, :],
                )
                sts.append(st)

            # per-supertile stats (column s of each is constant across
            # partitions after the GPSIMD all-reduce) and context partials
            SM = small.tile([P, NS], f32, tag="SM")    # local maxes
            SZ = small.tile([P, NS], f32, tag="SZ")    # per-partition sum-of-exp
            Csup = small.tile([NS, D], f32, tag="Csup")  # partial contexts

            for s in range(NS):
                # scores for this supertile
                S = small.tile([P, SUP], f32, tag="S")
                prod = prod_pool.tile([P, SUP, D], PHASE2_DT, tag="prod")
                nc.vector.tensor_tensor(
                    out=prod,
                    in0=sts[s],
                    in1=dec_bc3,
                    op=mybir.AluOpType.mult,
                )
                on_dve = s in DVE_REDUCE_SET
                if on_dve:
                    nc.vector.tensor_reduce(
                        out=S,
                        in_=prod,
                        axis=mybir.AxisListType.X,
                        op=mybir.AluOpType.add,
                    )
                else:
                    for jj in range(SUP):
                        junk = small.tile([P, D], PHASE2_DT, tag="junk")
                        nc.scalar.activation(
                            out=junk,
                            in_=prod[:, jj, :],
                            func=mybir.ActivationFunctionType.Copy,
                            bias=0.0,
                            scale=1.0,
                            accum_out=S[:, jj : jj + 1],
                        )

                # local softmax stats
                m_loc = small.tile([P, 1], f32, tag="m_loc")
                nc.vector.tensor_reduce(
                    out=m_loc, in_=S, axis=mybir.AxisListType.X,
                    op=mybir.AluOpType.max,
                )
                nc.gpsimd.partition_all_reduce(
                    SM[:, s : s + 1], m_loc, channels=P,
                    reduce_op=bass_isa.ReduceOp.max,
                )
                negm = small.tile([P, 1], f32, tag="negm")
                nc.gpsimd.tensor_scalar_mul(
                    out=negm, in0=SM[:, s : s + 1], scalar1=-1.0
                )

                probs = small.tile([P, SUP], PHASE2_DT, tag="probs")
                nc.scalar.activation(
                    out=probs,
                    in_=S,
                    func=mybir.ActivationFunctionType.Exp,
                    bias=negm,
                    scale=1.0,
                    accum_out=SZ[:, s : s + 1],
                )

                # partial context for this supertile
                ps = psum_c.tile([1, D], f32, tag="ps")
                for jj in range(SUP):
                    nc.tensor.matmul(
                        out=ps,
                        lhsT=probs[:, jj : jj + 1],
                        rhs=sts[s][:, jj, :],
                        start=(jj == 0),
                        stop=(jj == SUP - 1),
                    )
                # stage the partial at partition 0 (engines can't start at
                # partition s), then DMA it into row s of Csup
                csb = small.tile([1, D], f32, tag="csb")
                nc.vector.tensor_copy(out=csb, in_=ps)
                nc.sync.dma_start(out=Csup[s : s + 1, :], in_=csb)

            # combine: c = sum_s exp(m_s - M) * Csup[s] / sum_s exp(m_s - M) * Z_s
            M = small.tile([1, 1], f32, tag="M")
            nc.vector.tensor_reduce(
                out=M, in_=SM[0:1, :], axis=mybir.AxisListType.X,
                op=mybir.AluOpType.max,
            )
            negM = small.tile([1, 1], f32, tag="negM")
            nc.gpsimd.tensor_scalar_mul(out=negM, in0=M, scalar1=-1.0)
            w_row = small.tile([1, NS], f32, tag="w_row")
            nc.scalar.activation(
                out=w_row,
                in_=SM[0:1, :],
                func=mybir.ActivationFunctionType.Exp,
                bias=negM,
                scale=1.0,
            )
            # Z_col[s] = sum_p SZ[p, s] via PE, then Z = w . Z_col
            ps_z = psum_w.tile([NS, 1], f32, tag="ps_z")
            nc.tensor.matmul(
                out=ps_z, lhsT=SZ, rhs=ones_col, start=True, stop=True
            )
            z_col = small.tile([NS, 1], f32, tag="z_col")
            nc.vector.tensor_copy(out=z_col, in_=ps_z)

            # w as a column via PE transpose, then c_hat = w^T @ Csup
            ps_w = psum_w.tile([NS, 1], f32, tag="ps_w")
            nc.tensor.transpose(out=ps_w, in_=w_row, identity=ident1)
            w_col = small.tile([NS, 1], f32, tag="w_col")
            nc.vector.tensor_copy(out=w_col, in_=ps_w)
            ps_zf = psum_w.tile([1, 1], f32, tag="ps_zf")
            nc.tensor.matmul(
                out=ps_zf, lhsT=w_col, rhs=z_col, start=True, stop=True
            )
            invz = small.tile([1, 1], f32, tag="invz")
            nc.vector.reciprocal(out=invz, in_=ps_zf)
            ps_c = psum_w.tile([1, D], f32, tag="ps_chat")
            nc.tensor.matmul(
                out=ps_c, lhsT=w_col, rhs=Csup, start=True, stop=True
            )

            c_sb = outp.tile([1, D], f32, tag="c_sb")
            nc.vector.tensor_scalar_mul(out=c_sb, in0=ps_c, scalar1=invz)
            nc.sync.dma_start(out=out_ap[b : b + 1, :], in_=c_sb)

    nc.compile()
    return nc


_NC_CACHE = None


def _get_nc():
    global _NC_CACHE
    if _NC_CACHE is None:
        _NC_CACHE = _build_nc()
    return _NC_CACHE


def run_on_cores(enc_np: np.ndarray, dec_np: np.ndarray, trace: bool = False):
    """Returns (out [32, 256] f32, BassKernelResults)."""
    nc = _get_nc()
    in_maps = [
        {
            "enc": np.ascontiguousarray(enc_np[c * B_LOC : (c + 1) * B_LOC]),
            "dec": np.ascontiguousarray(dec_np[c * B_LOC : (c + 1) * B_LOC]),
        }
        for c in range(N_CORES)
    ]
    res = run_bass_kernel_spmd(nc, in_maps, list(range(N_CORES)), trace=trace)
    out = np.concatenate([r["out"] for r in res.results], axis=0)
    return out.astype(np.float32), res


def kernel(enc_hid_states, dec_hid):
    enc_np = np.asarray(enc_hid_states, dtype=np.float32)
    dec_np = np.asarray(dec_hid, dtype=np.float32)
    out, _ = run_on_cores(enc_np, dec_np, trace=False)
    return out
